# revision 21
# baseline (speedup 1.0000x reference)
"""Trainium2 Bass kernel for nn_GAT_27960237097248.

The reference network's output is tanh(edges) after two *edge* GAT layers;
the node path never feeds back into edges (dead code).  For the edge layers
(num_heads=1) the source bug `split = a.shape[0]//2 == 0` makes lp == 0 and
lc[j] = H[k,j] * sum(a), so per batch b and edge-slice k the masked softmax
over j collapses algebraically:

    Z    = X @ Wadj                       (X = edges[b], badj is zero)
    Zsym = Z + Z^T                        (sigmoid(x)+sigmoid(y) > 1  <=>  x+y > 0)
    adj  = (Zsym > 0)                     (symmetric 0/1 mask)
    H    = X @ Wp
    E    = exp(leaky_relu(S*H, 0.2))      (S = sum(a); no row-max needed: |L| <= ~10)
    out  = ((E*H) @ adj) / (E @ adj)      (adj symmetric, exp(NEG)==0)
    X'   = (out + out^T) / 2              (0.5 folded into next layer's weights)

Final output: tanh(0.5*(out + out^T)) after layer 1.

Scheduling/precision strategy (v5, from the v2/v3 traces + numpy
precision decomposition):
  * Input DMA wire time is the startup bottleneck (8 cores load
    simultaneously; fp32 x wasn't resident until ~15us), and fp32
    matmuls cost 2 PE MATMUL instructions each (LOW/HIGH passes).
    Layer-0's Zsym threshold is hypersensitive to rounding of X
    (fp16 X -> 1.6e-2, at the gate), so x/wadj0 stay fp32 -- but
    everything else measured harmless in fp16 (<= 3e-3 total): the H
    chains (fp16 x copy + fp16 wp), layer-1's whole Zsym chain (X'
    written as fp16 directly by the DVE symmetrize add, wadj1 fp16,
    single-pass Z1 matmuls).  Ships 1.02MB instead of 1.28MB and
    cuts the PE instruction count by ~20.
  * Zsym = Z + Z^T is accumulated directly in PSUM (Z: stationary = X^T
    chunks, moving = Wadj rows; Z^T: stationary = Wadj column chunks,
    moving = X^T rows).  No PSUM->SBUF copy, no PE transposes, no
    identity matrix.  adj = (Zsym > 0) is one DVE tensor_scalar.
  * num/den are computed in BOTH orientations as matmuls from adj and
    E/EH (adj is symmetric), removing output-side transposes.  den
    matmuls go to their own PSUM tile and are issued first, so ln(den)
    starts while the num matmuls still run.
  * Division = exp(-ln(den)); exp/ln/parametric_relu all live in one
    activation table set (natural_log_exp_and_others, id 6) loaded
    manually at t=0.  The compiler's per-function placement would thrash
    exp_and_others <-> natural_log 4x mid-kernel (~2.7us each).  The
    only remaining switch (Tanh, set 0) is triggered by a dummy right
    after the last exp and overlaps the final DVE multiply/add.
  * leaky_relu(S*H) = Prelu with the S fold in the activation scale.
  * X' for layer 1 is written as fp16 directly by the DVE symmetrize
    add (no separate cast).
  * PE warmup junk matmuls feed off a Vector-engine memset so they start
    during the DMA ramp (HAM clock gate).

Core c computes batch c % 4 end-to-end (batches are independent).
"""

import numpy as np

_N = 256
_P = 128
_B = 4
_NCORES = 8


def _build_program(s_nonpos=(True, True)):
    """Build the single-core Bass program (shared SPMD across all cores).

    The program is fully data-independent (s_nonpos is accepted for
    interface compatibility but unused: Prelu(S*H, 0.2) handles either
    sign of S exactly).  All runtime data arrives via ExternalInput
    dram tensors.
    """
    import concourse.tile as tile
    from concourse import bacc, mybir

    f32 = mybir.dt.float32
    f16 = mybir.dt.float16
    bf16 = mybir.dt.bfloat16
    AF = mybir.ActivationFunctionType
    OP = mybir.AluOpType

    nc = bacc.Bacc(
        "TRN2", target_bir_lowering=False, debug=False, enable_asserts=False
    )

    # ---- DRAM I/O (per-core) ----
    edges_t = nc.dram_tensor("edges_t", [2, _P, _N], f32, kind="ExternalInput")
    edges_th = nc.dram_tensor("edges_th", [2, _P, _N], f16, kind="ExternalInput")
    wadj_d = [
        nc.dram_tensor("wadj0", [2, _P, _N], f32, kind="ExternalInput"),
        nc.dram_tensor("wadj1", [2, _P, _N], f16, kind="ExternalInput"),
    ]
    wp_d = [
        nc.dram_tensor(f"wp{l}", [2, _P, _N], f16, kind="ExternalInput")
        for l in (0, 1)
    ]
    svec_d = nc.dram_tensor("svec", [2, _P, 1], f32, kind="ExternalInput")
    out_d = nc.dram_tensor("out", [2, _P, _N], f32, kind="ExternalOutput")

    with tile.TileContext(nc) as tc:
        with (
            nc.allow_low_precision("fp16/bf16 chains verified vs the 2e-2 gate"),
            tc.tile_pool(name="const", bufs=1) as cp,
            tc.tile_pool(name="work", bufs=2) as sp,
            tc.tile_pool(name="psum", bufs=1, space="PSUM") as pp,
        ):
            # ---- tiles ----
            x = sp.tile([_P, 2 * _N], f32, tag="x")
            xh = cp.tile([_P, 2 * _N], f16, tag="xh")
            wadj_t = [cp.tile([_P, 2 * _N], f32, tag="wadj0", name="wadj_t0"),
                      cp.tile([_P, 2 * _N], f16, tag="wadj1", name="wadj_t1")]
            wp_t = [cp.tile([_P, 2 * _N], f16, tag=f"wp{l}", name=f"wp_t{l}")
                    for l in (0, 1)]
            s_ap = [cp.tile([_P, 1], f32, tag=f"svec{l}", name=f"s_ap{l}")
                    for l in (0, 1)]
            junkb = cp.tile([_P, 2 * _N], bf16, tag="junkb")
            epsb = cp.tile([_P, 1], f32, tag="epsb")

            # ---- activation tables: load natural_log_exp_and_others (6)
            # once at t=0; every prelu/exp/ln below then runs with zero
            # table switches. ----
            nc.scalar.add_instruction(
                mybir.InstLoadActFuncSet(
                    name=nc.get_next_instruction_name(),
                    ins=[],
                    outs=[],
                    act_func_set_id=6,
                )
            )

            # ---- input DMAs, one 3D descriptor each.  Layer-0 tensors on
            # the HWDGE queues (sync/scalar, fast completion); gpsimd SWDGE
            # semaphores post ~5us late so it only carries layer-1 weights.
            # svec first on scalar: prelu needs it early and it's tiny. ----
            def dma3d(eng, tile_, dram_):
                eng.dma_start(
                    tile_[:].rearrange("p (b n) -> p b n", b=2),
                    dram_[:].rearrange("b p n -> p b n"),
                )

            nc.scalar.dma_start(s_ap[0][:], svec_d[0])
            nc.scalar.dma_start(s_ap[1][:], svec_d[1])
            dma3d(nc.scalar, xh, edges_th)
            dma3d(nc.scalar, wp_t[0], wp_d[0])
            dma3d(nc.sync, x, edges_t)
            dma3d(nc.sync, wadj_t[0], wadj_d[0])
            nc.vector.memset(junkb[:], 0.0)
            nc.vector.memset(epsb[:], 1e-20)
            dma3d(nc.gpsimd, wadj_t[1], wadj_d[1])
            dma3d(nc.gpsimd, wp_t[1], wp_d[1])

            mm = nc.tensor.matmul

            # ---- PE warmup: dep-free junk matmuls (input is the vector
            # memset, available immediately) keep the HAM activity monitor
            # busy during the DMA ramp so real matmuls run at 2.4 GHz ----
            junkp = pp.tile([_P, 2 * _N], f32, tag="junk")
            for w in range(10):
                mm(
                    junkp[:],
                    junkb[:, 0:_P],
                    junkb[:],
                    start=(w == 0),
                    stop=(w == 9),
                )

            for l in (0, 1):
                # hx: fp16 X for the H matmuls; zx: X for the Zsym matmuls
                # (layer 0 fp32 -- its threshold is precision-critical;
                #  layer 1 fp16 X' -- measured harmless).
                hx = xh if l == 0 else x
                zx = x
                # H^T = (X @ Wp)^T, fp16 single-pass.
                ht = pp.tile([_P, 2 * _N], f32, tag="ht")
                for p in (0, 1):
                    for kc in (0, 1):
                        mm(
                            ht[:, p * _N : (p + 1) * _N],
                            wp_t[l][:, kc * _N + p * _P : kc * _N + (p + 1) * _P],
                            hx[:, kc * _N : (kc + 1) * _N],
                            start=(kc == 0),
                            stop=(kc == 1),
                        )
                # leaky_relu(S*H) = Prelu(S*H, 0.2) -- in table set 6, so
                # no switch; S is the activation input scale.
                lt = sp.tile([_P, 2 * _N], f32, tag="lt")
                nc.scalar.activation(
                    lt[:], ht[:], AF.Prelu, scale=s_ap[l][:], alpha=0.2
                )
                eeE = sp.tile([_P, 2 * _N], bf16, tag="eeE")
                nc.scalar.activation(eeE[:], lt[:], AF.Exp)
                # Zsym = Z + Z^T accumulated in one PSUM tile.
                zsym = pp.tile([_P, 2 * _N], f32, tag="zsym")
                for p in (0, 1):
                    dst = zsym[:, p * _N : (p + 1) * _N]
                    for kc in (0, 1):  # Z rows:  X^T chunks x Wadj rows
                        mm(
                            dst,
                            zx[:, kc * _N + p * _P : kc * _N + (p + 1) * _P],
                            wadj_t[l][:, kc * _N : (kc + 1) * _N],
                            start=(kc == 0),
                            stop=False,
                        )
                    for kc in (0, 1):  # Z^T rows: Wadj col-chunks x X^T rows
                        mm(
                            dst,
                            wadj_t[l][:, kc * _N + p * _P : kc * _N + (p + 1) * _P],
                            zx[:, kc * _N : (kc + 1) * _N],
                            start=False,
                            stop=(kc == 1),
                        )
                # adj first on the DVE (zsym is ready before exp's output),
                # then the EH multiply.
                adj = sp.tile([_P, 2 * _N], bf16, tag="adj")
                nc.vector.tensor_scalar(adj[:], zsym[:], 0.0, None, OP.is_gt)
                eeEH = sp.tile([_P, 2 * _N], bf16, tag="eeEH")
                nc.vector.tensor_tensor(eeEH[:], eeE[:], ht[:], OP.mult)

                # ndd = [den^T | den], ndn = [num^T | num]: separate PSUM
                # tiles so ln only waits on the 8 den matmuls and overlaps
                # the num matmuls.
                ndd = pp.tile([_P, 4 * _N], f32, tag="ndd")
                ndn = pp.tile([_P, 4 * _N], f32, tag="ndn")
                for dst, off, lhs, rhs in (
                    (ndd, 0, adj, eeE),        # den^T: adj chunks  x E rows
                    (ndd, 2 * _N, eeE, adj),   # den:   E^T chunks  x adj rows
                    (ndn, 0, adj, eeEH),       # num^T: adj chunks  x EH rows
                    (ndn, 2 * _N, eeEH, adj),  # num:   EH^T chunks x adj rows
                ):
                    for p in (0, 1):
                        for jc in (0, 1):
                            mm(
                                dst[:, off + p * _N : off + (p + 1) * _N],
                                lhs[:, jc * _N + p * _P : jc * _N + (p + 1) * _P],
                                rhs[:, jc * _N : (jc + 1) * _N],
                                start=(jc == 0),
                                stop=(jc == 1),
                            )

                # 1/den = exp(-ln(den + 1e-20)) -- all in table set 6.
                # The epsilon guards rows whose adjacency went all-zero
                # (possible via precision flips): out-row becomes 0
                # instead of inf/NaN.
                rec = sp.tile([_P, 4 * _N], f32, tag="rec")
                nc.scalar.activation(rec[:], ndd[:], AF.Ln, bias=epsb[:])
                rec2 = sp.tile([_P, 4 * _N], f32, tag="rec2")
                nc.scalar.activation(rec2[:], rec[:], AF.Exp, scale=-1.0)
                outv = sp.tile([_P, 4 * _N], f32, tag="outv")
                nc.vector.tensor_tensor(outv[:], ndn[:], rec2[:], OP.mult)

                # symmetrize: [out^T | out] halves are block-aligned, so
                # X' (fp16, consumed by layer 1's matmuls directly) or the
                # fp32 tanh input is one contiguous DVE add.
                if l == 0:
                    # inter-layer junk bridge: keep the PE busy through
                    # the division tail so the HAM clock stays at 2.4GHz
                    # for layer 1's matmuls (v5 ran all of layer 1 cold).
                    for w in range(26):
                        mm(
                            junkp[:, 0:_N],
                            junkb[:, 0:_P],
                            junkb[:, 0:_N],
                            start=(w == 0),
                            stop=(w == 25),
                        )
                    x = sp.tile([_P, 2 * _N], f16, tag="x")
                    nc.vector.tensor_tensor(
                        x[:], outv[:, 0 : 2 * _N], outv[:, 2 * _N : 4 * _N],
                        OP.add,
                    )
                else:
                    # dummy tanh right after the last exp: triggers the
                    # single table switch (set 0) under the DVE mult/add.
                    dummyt = sp.tile([_P, 1], f32, tag="dummy")
                    nc.scalar.activation(dummyt[:], rec2[:, 0:1], AF.Tanh)
                    v = sp.tile([_P, 2 * _N], f32, tag="v")
                    nc.vector.tensor_tensor(
                        v[:], outv[:, 0 : 2 * _N], outv[:, 2 * _N : 4 * _N],
                        OP.add,
                    )
                    res = sp.tile([_P, 2 * _N], f32, tag="res")
                    # split tanh + store per block so block 0's DMA overlaps
                    # block 1's tanh
                    for p in (0, 1):
                        nc.scalar.activation(
                            res[:, p * _N : (p + 1) * _N],
                            v[:, p * _N : (p + 1) * _N],
                            AF.Tanh,
                            scale=0.5,
                        )
                        nc.sync.dma_start(out_d[p], res[:, p * _N : (p + 1) * _N])

    nc.compile()
    return nc


def _make_in_maps(inputs):
    """Host-side prep: fold constants, transpose edges, build per-core maps."""
    edges = np.ascontiguousarray(np.asarray(inputs["edges"], dtype=np.float32))
    assert edges.shape == (_B, _N, _N)

    wadj = [np.asarray(inputs["wadj_e0"], np.float32),
            np.asarray(inputs["wadj_e1"], np.float32)]
    wp = [np.asarray(inputs["wp_e0"], np.float32),
          np.asarray(inputs["wp_e1"], np.float32)]
    s = [float(np.asarray(inputs["a_e0"]).astype(np.float64).sum()),
         float(np.asarray(inputs["a_e1"]).astype(np.float64).sum())]
    for key in ("badj_e0", "badj_e1", "bp_e0", "bp_e1"):
        assert not np.any(np.asarray(inputs[key])), f"nonzero bias {key} unsupported"

    # 0.5 symmetrize factor of layer 0's output folded into layer 1 weights
    wadj[1] = wadj[1] * 0.5
    wp[1] = wp[1] * 0.5

    common = {
        "wadj0": np.ascontiguousarray(wadj[0].reshape(2, _P, _N)),
        "wadj1": np.ascontiguousarray(
            wadj[1].reshape(2, _P, _N).astype(np.float16)
        ),
        "wp0": np.ascontiguousarray(wp[0].reshape(2, _P, _N).astype(np.float16)),
        "wp1": np.ascontiguousarray(wp[1].reshape(2, _P, _N).astype(np.float16)),
        "svec": np.stack(
            [np.full((_P, 1), s[0], np.float32), np.full((_P, 1), s[1], np.float32)]
        ),
    }

    in_maps = []
    for c in range(_NCORES):
        b = c % _B
        m = dict(common)
        et = np.ascontiguousarray(edges[b].T.reshape(2, _P, _N))
        m["edges_t"] = et
        m["edges_th"] = np.ascontiguousarray(et.astype(np.float16))
        in_maps.append(m)
    return in_maps


def kernel(**inputs):
    import sys
    if not any("trn_rl_repo" in p for p in sys.path):
        sys.path.insert(0, "/opt/trn_rl_repo")
    from concourse.bass_utils import run_bass_kernel_spmd

    s_nonpos = tuple(
        float(np.asarray(inputs[k]).sum()) <= 0 for k in ("a_e0", "a_e1")
    )
    nc = _build_program(s_nonpos)
    in_maps = _make_in_maps(inputs)
    res = run_bass_kernel_spmd(nc, in_maps, core_ids=list(range(_NCORES)))

    outs = []
    for b in range(_B):
        o = res.results[b]["out"]  # [2, 128, 256]
        outs.append(np.concatenate([o[0], o[1]], axis=0))
    full = np.ascontiguousarray(np.stack(outs).astype(np.float32))
    return full, full
